# revision 1
# baseline (speedup 1.0000x reference)
"""CompressionHead kernel for Trainium2 (8 NeuronCores, Bass/Tile).

Reference computes:
    u          = h / max(||h||_2, eps)              (row-normalize, dim=-1)
    sim        = einsum('bid,bjd->bij', u, u)       (B,S,S) batched GEMM
    conc       = (sum(sim) - trace(sim)) / (B*S*(S-1))
    lambda_t   = sigmoid(alpha * (conc - beta))
    returns (lambda_t, conc)

Key identity: sum_{i,j} u_i . u_j = || sum_i u_i ||^2, so the O(B*S^2*D)
GEMM collapses to an O(B*S*D) normalize-and-reduce:
    sum(sim)   = sum_b || s_b ||^2,   s_b = sum_i u_{b,i}
    trace(sim) = sum_{b,i} u_{b,i} . u_{b,i}   (~= B*S)

Sharding: flatten (B,S) -> 16384 rows; each of the 8 cores takes a
contiguous 2048-row block (2 cores per batch; blocks never straddle a
batch). Per core, per [128, 2048] row-tile:
  - row sum-of-squares ss split between ACT (Square+accum, cols 0..1279)
    and DVE (mul+reduce, cols 1280..2047; DVE needs 2 passes so it gets
    the smaller share) so neither engine exceeds the DMA-bound tile time
  - ACT sqrt + DVE reciprocal: inv[p] = 1/||x_p||
  - PE matmul psum[1,:] += inv.T @ x accumulated over the 16 row-tiles
    (u never materialized; inv is folded into the MAC). Operands typed
    float32r: full-rate PE (1 cyc/row vs 4 for f32) at slightly relaxed
    precision — measured conc rel err 5.8e-4 vs the f32 reference.
  - diag column: ss * inv^2  (per-row u.u, matches reference to f32 noise)
Host combines the tiny per-core outputs in float64.

Timing (cost-model TimelineSim, per core): 56.4us against a 50.1us
pure-DMA floor (16.78MB @ 360GB/s + fixed drain) — the residual is the
last tile's inv chain + stop-matmul + PSUM-drain tail, each individually
minimized (sqrt-bias fold, latency-balanced last-tile split, ACT/DVE
split of the PSUM copies).
"""

import numpy as np

B, S, D = 4, 4096, 2048
N_CORES = 8
ROWS_PER_CORE = (B * S) // N_CORES  # 2048
P = 128
N_TILES = ROWS_PER_CORE // P  # 16
N_CHUNK = 512  # PSUM-bank / fp32 moving-operand limit per matmul
N_CHUNKS = D // N_CHUNK  # 4

MM_F32R = True  # PE matmul in float32r (full rate vs 4 cyc/row for f32)

_CACHE = {}


def _build_nc():
    import concourse.tile as tile
    from concourse import bacc, mybir

    F32 = mybir.dt.float32
    F32R = mybir.dt.float32r
    nc = bacc.Bacc(None, target_bir_lowering=False, debug=True)
    x = nc.dram_tensor("x", [ROWS_PER_CORE, D], F32, kind="ExternalInput")
    s_out = nc.dram_tensor("s_out", [1, D], F32, kind="ExternalOutput")
    d_out = nc.dram_tensor("d_out", [P, N_TILES], F32, kind="ExternalOutput")

    with tile.TileContext(nc) as tc:
        with (
            tc.tile_pool(name="xp", bufs=4) as xp,
            tc.tile_pool(name="scratch", bufs=2) as scratch,
            tc.tile_pool(name="small", bufs=4) as small,
            tc.tile_pool(name="psum", bufs=1, space="PSUM") as pp,
            tc.tile_pool(name="outp", bufs=1) as outp,
        ):
            psums = [
                pp.tile([1, N_CHUNK], F32, name=f"ps{k}", tag=f"ps{k}")
                for k in range(N_CHUNKS)
            ]
            d_sb = outp.tile([P, N_TILES], F32, name="d_sb")
            s_sb = outp.tile([1, D], F32, name="s_sb")

            for t in range(N_TILES):
                last = t == N_TILES - 1
                # ACT's column share; DVE runs 2 passes (mul+reduce) on the
                # rest. The last tile sits on the kernel's critical tail, so
                # it gets a latency-balanced split instead of the
                # throughput-balanced one.
                H = 1792 if last else 1280
                # xt typed f32r so the BIR verifier accepts it as an f32r
                # matmul operand (same 4-byte payload as f32); non-matmul
                # consumers read it bitcast back to f32.
                xt_dt = F32R if MM_F32R else F32
                xt = xp.tile([P, D], xt_dt, name="xt", tag="xt")
                src = x[t * P : (t + 1) * P, :]
                if MM_F32R:
                    src = src.bitcast(F32R)
                if last:
                    # Split the final DMA so ACT's (larger) share lands
                    # first and its reduction overlaps the in-flight rest —
                    # this DMA ends the kernel's critical chain.
                    nc.sync.dma_start(out=xt[:, D - H :], in_=src[:, D - H :])
                    nc.sync.dma_start(out=xt[:, : D - H], in_=src[:, : D - H])
                    act_sl = slice(D - H, D)
                    dve_sl = slice(0, D - H)
                else:
                    nc.sync.dma_start(out=xt[:], in_=src)
                    act_sl = slice(0, H)
                    dve_sl = slice(H, D)
                xtf = xt[:].bitcast(F32) if MM_F32R else xt[:]

                # ss[p] = sum_d xt[p,d]^2, halves on ACT and DVE in parallel
                # (sqa/sqb are throwaway full-width outputs the ISA requires)
                sqa = scratch.tile([P, H], F32, name="sqa", tag="sqa")
                ssa = small.tile([P, 1], F32, name="ssa", tag="ssa")
                nc.scalar.activation(
                    sqa[:],
                    xtf[:, act_sl],
                    mybir.ActivationFunctionType.Square,
                    accum_out=ssa[:],
                )
                # DVE half: tensor_tensor_reduce would fuse these two, but
                # that extended op crashes the NEFF at runtime on this stack
                # (its DVE ucode table isn't delivered) — use plain ops.
                sqb = scratch.tile([P, D - H], F32, name="sqb", tag="sqb")
                nc.vector.tensor_mul(sqb[:], xtf[:, dve_sl], xtf[:, dve_sl])
                ssb = small.tile([P, 1], F32, name="ssb", tag="ssb")
                nc.vector.tensor_reduce(
                    ssb[:],
                    sqb[:],
                    axis=mybir.AxisListType.X,
                    op=mybir.AluOpType.add,
                )
                # fold the halves-combine into sqrt's bias operand:
                # nrm = Sqrt(ssa*1 + ssb) — one hop shorter critical path
                nrm = small.tile([P, 1], F32, name="nrm", tag="nrm")
                nc.scalar.activation(
                    nrm[:],
                    ssa[:],
                    mybir.ActivationFunctionType.Sqrt,
                    bias=ssb[:],
                )
                # ss (= ssa+ssb) still needed for the diag column, off the
                # critical path
                ss = small.tile([P, 1], F32, name="ss", tag="ss")
                nc.vector.tensor_add(ss[:], ssa[:], ssb[:])
                inv = small.tile([P, 1], xt_dt, name="inv", tag="inv")
                if MM_F32R:
                    with nc.allow_low_precision(reason="f32r keeps f32 width"):
                        nc.vector.reciprocal(inv[:], nrm[:])
                else:
                    nc.vector.reciprocal(inv[:], nrm[:])
                invf = inv[:].bitcast(F32) if MM_F32R else inv[:]

                # diag contribution of each row: ss * inv^2 == u . u
                ssi = small.tile([P, 1], F32, name="ssi", tag="ssi")
                nc.vector.tensor_mul(ssi[:], ss[:], invf)
                nc.vector.tensor_mul(d_sb[:, t : t + 1], ssi[:], invf)

                for k in range(N_CHUNKS):
                    nc.tensor.matmul(
                        psums[k][:],
                        lhsT=inv[:],
                        rhs=xt[:, k * N_CHUNK : (k + 1) * N_CHUNK],
                        start=(t == 0),
                        stop=(t == N_TILES - 1),
                    )

            # drain PSUM on both ACT and DVE so the tail halves
            for k in range(N_CHUNKS):
                dst = s_sb[:, k * N_CHUNK : (k + 1) * N_CHUNK]
                if k < 2:
                    nc.scalar.copy(dst, psums[k][:])
                else:
                    nc.vector.tensor_copy(dst, psums[k][:])
            nc.sync.dma_start(out=s_out[:], in_=s_sb[:])
            nc.sync.dma_start(out=d_out[:], in_=d_sb[:])

    # Full bacc lowering: splits multi-sem waits into event semaphores,
    # moves matmul waits onto LDWEIGHTS, populates extended-inst ISA bytes.
    # Raw Bass skips all of this and walrus codegen rejects the result.
    nc.compile()
    return nc


def get_nc():
    if "nc" not in _CACHE:
        _CACHE["nc"] = _build_nc()
    return _CACHE["nc"]


def make_in_maps(h):
    flat = np.ascontiguousarray(np.asarray(h, dtype=np.float32)).reshape(B * S, D)
    return [
        {"x": flat[c * ROWS_PER_CORE : (c + 1) * ROWS_PER_CORE]}
        for c in range(N_CORES)
    ]


def finish(results, alpha, beta):
    """Combine per-core partial outputs (host, float64)."""
    s_parts = np.stack([np.asarray(r["s_out"][0], dtype=np.float64) for r in results])
    diag = float(sum(np.asarray(r["d_out"], dtype=np.float64).sum() for r in results))
    cores_per_batch = N_CORES // B
    s_b = s_parts.reshape(B, cores_per_batch, D).sum(axis=1)  # (B, D)
    sum_sim = float((s_b * s_b).sum())
    denom = float(B) * S * (S - 1)
    conc = (sum_sim - diag) / denom
    lam = 1.0 / (1.0 + np.exp(-(float(alpha) * (conc - float(beta)))))
    return (
        np.asarray(lam, dtype=np.float32),
        np.asarray(conc, dtype=np.float32),
    )


def kernel(h, alpha, beta):
    import time

    from concourse.bass_utils import run_bass_kernel_spmd

    nc = get_nc()
    in_maps = make_in_maps(h)
    last_err = None
    for attempt in range(3):
        # The axon-tunneled device intermittently reports
        # NRT_EXEC_UNIT_UNRECOVERABLE on an otherwise-healthy NEFF; a
        # short-delay retry recovers it.
        try:
            results = run_bass_kernel_spmd(
                nc, in_maps, core_ids=list(range(N_CORES))
            ).results
            return finish(results, alpha, beta)
        except Exception as e:  # noqa: BLE001 - retry any device-side failure
            last_err = e
            time.sleep(5.0 * (attempt + 1))
    raise last_err



# revision 2
# speedup vs baseline: 1.0052x; 1.0052x over previous
"""CompressionHead kernel for Trainium2 (8 NeuronCores, Bass/Tile).

Reference computes:
    u          = h / max(||h||_2, eps)              (row-normalize, dim=-1)
    sim        = einsum('bid,bjd->bij', u, u)       (B,S,S) batched GEMM
    conc       = (sum(sim) - trace(sim)) / (B*S*(S-1))
    lambda_t   = sigmoid(alpha * (conc - beta))
    returns (lambda_t, conc)

Key identity: sum_{i,j} u_i . u_j = || sum_i u_i ||^2, so the O(B*S^2*D)
GEMM collapses to an O(B*S*D) normalize-and-reduce:
    sum(sim)   = sum_b || s_b ||^2,   s_b = sum_i u_{b,i}
    trace(sim) = sum_{b,i} u_{b,i} . u_{b,i} = B*S exactly (u is unit-norm;
                 ||h|| ~ sqrt(D) >> eps), so it is not computed on device.

Sharding: flatten (B,S) -> 16384 rows; each of the 8 cores takes a
contiguous 2048-row block (2 cores per batch; blocks never straddle a
batch). Per core, per [128, 2048] row-tile:
  - row sum-of-squares split between ACT (Square+accum) and DVE
    (mul+reduce; DVE needs 2 passes so it gets the smaller share)
  - nrm = Sqrt(ssa + bias=ssb) on ACT, inv = 1/nrm on DVE
  - PE matmul psum[1,:] += inv.T @ x accumulated over the 16 row-tiles
    (u never materialized; inv is folded into the MAC). Operands typed
    float32r: full-rate PE (1 cyc/col vs 4 for f32).
The last tile's DMA is split in three so only a 128-column chunk lands
last; its Square(+accum) is the only norm work left after the HBM
stream drains, then Sqrt(bias-folded partials) -> reciprocal -> the four
stop-matmuls -> per-bank PSUM->SBUF copies (interleaved ACT/DVE, each
gated only on its own bank's matmul) -> one 8KB output DMA.
Host combines the tiny per-core outputs in float64.
"""

import numpy as np

B, S, D = 4, 4096, 2048
N_CORES = 8
ROWS_PER_CORE = (B * S) // N_CORES  # 2048
P = 128
N_TILES = ROWS_PER_CORE // P  # 16
N_CHUNK = 512  # PSUM-bank / fp32 moving-operand limit per matmul
N_CHUNKS = D // N_CHUNK  # 4

MM_F32R = True  # PE matmul in float32r (full rate vs 4 cyc/row for f32)

# Last tile's column split: ACT does [0:A1]+[A1:A2] as two Square+accum
# passes (pipelined with the arriving sub-DMAs), DVE does [A2:DV], and the
# final 128-col chunk [DV:D] is the only work gated on the last DMA.
LT_A1 = 640
LT_A2 = 1280
LT_DV = 1920

_CACHE = {}


def _build_nc():
    import concourse.tile as tile
    from concourse import bacc, mybir

    F32 = mybir.dt.float32
    F32R = mybir.dt.float32r
    nc = bacc.Bacc(None, target_bir_lowering=False, debug=True)
    x = nc.dram_tensor("x", [ROWS_PER_CORE, D], F32, kind="ExternalInput")
    s_out = nc.dram_tensor("s_out", [1, D], F32, kind="ExternalOutput")

    with tile.TileContext(nc) as tc:
        with (
            tc.tile_pool(name="xp", bufs=4) as xp,
            tc.tile_pool(name="scratch", bufs=2) as scratch,
            tc.tile_pool(name="small", bufs=4) as small,
            tc.tile_pool(name="psum", bufs=1, space="PSUM") as pp,
            tc.tile_pool(name="outp", bufs=1) as outp,
        ):
            psums = [
                pp.tile([1, N_CHUNK], F32, name=f"ps{k}", tag=f"ps{k}")
                for k in range(N_CHUNKS)
            ]
            s_sb = outp.tile([1, D], F32, name="s_sb")

            xt_dt = F32R if MM_F32R else F32

            for t in range(N_TILES):
                last = t == N_TILES - 1
                # xt typed f32r so the BIR verifier accepts it as an f32r
                # matmul operand (same 4-byte payload as f32); non-matmul
                # consumers read it bitcast back to f32.
                xt = xp.tile([P, D], xt_dt, name="xt", tag="xt")
                src = x[t * P : (t + 1) * P, :]
                if MM_F32R:
                    src = src.bitcast(F32R)
                xtf = xt[:].bitcast(F32) if MM_F32R else xt[:]

                if not last:
                    nc.sync.dma_start(out=xt[:], in_=src)
                    # ss[p] = sum_d xt[p,d]^2; ACT takes the larger share
                    # (one pass: Square+accum), DVE the smaller (two
                    # passes: mul then reduce). tensor_tensor_reduce would
                    # fuse DVE's two, but that extended op crashes the NEFF
                    # at runtime on this stack.
                    H = 1280
                    sqa = scratch.tile([P, H], F32, name="sqa", tag="sqa")
                    ssa = small.tile([P, 1], F32, name="ssa", tag="ssa")
                    nc.scalar.activation(
                        sqa[:],
                        xtf[:, 0:H],
                        mybir.ActivationFunctionType.Square,
                        accum_out=ssa[:],
                    )
                    sqb = scratch.tile([P, D - H], F32, name="sqb", tag="sqb")
                    nc.vector.tensor_mul(sqb[:], xtf[:, H:D], xtf[:, H:D])
                    ssb = small.tile([P, 1], F32, name="ssb", tag="ssb")
                    nc.vector.tensor_reduce(
                        ssb[:],
                        sqb[:],
                        axis=mybir.AxisListType.X,
                        op=mybir.AluOpType.add,
                    )
                    # fold the halves-combine into sqrt's bias operand:
                    # nrm = Sqrt(ssa*1 + ssb)
                    nrm = small.tile([P, 1], F32, name="nrm", tag="nrm")
                    nc.scalar.activation(
                        nrm[:],
                        ssa[:],
                        mybir.ActivationFunctionType.Sqrt,
                        bias=ssb[:],
                    )
                else:
                    # Last tile: stream the columns in four sub-DMAs so the
                    # row sum-of-squares is nearly done when the final
                    # 128-col chunk lands. Shares: ACT [0:A1]+[A1:A2] (two
                    # Square+accum passes) and the final [DV:D]; DVE
                    # [A2:DV] (mul+reduce).
                    nc.sync.dma_start(out=xt[:, LT_A2:LT_DV], in_=src[:, LT_A2:LT_DV])
                    nc.sync.dma_start(out=xt[:, 0:LT_A1], in_=src[:, 0:LT_A1])
                    nc.sync.dma_start(out=xt[:, LT_A1:LT_A2], in_=src[:, LT_A1:LT_A2])
                    nc.sync.dma_start(out=xt[:, LT_DV:D], in_=src[:, LT_DV:D])

                    sqb = scratch.tile([P, LT_DV - LT_A2], F32, name="sqb", tag="sqb")
                    nc.vector.tensor_mul(
                        sqb[:], xtf[:, LT_A2:LT_DV], xtf[:, LT_A2:LT_DV]
                    )
                    ssb = small.tile([P, 1], F32, name="ssb", tag="ssb")
                    nc.vector.tensor_reduce(
                        ssb[:],
                        sqb[:],
                        axis=mybir.AxisListType.X,
                        op=mybir.AluOpType.add,
                    )
                    sqa1 = scratch.tile([P, LT_A1], F32, name="sqa1", tag="sqa")
                    ssa1 = small.tile([P, 1], F32, name="ssa1", tag="ssa1")
                    nc.scalar.activation(
                        sqa1[:],
                        xtf[:, 0:LT_A1],
                        mybir.ActivationFunctionType.Square,
                        accum_out=ssa1[:],
                    )
                    sqa2 = scratch.tile([P, LT_A2 - LT_A1], F32, name="sqa2", tag="sqa2")
                    ssa2 = small.tile([P, 1], F32, name="ssa2", tag="ssa2")
                    nc.scalar.activation(
                        sqa2[:],
                        xtf[:, LT_A1:LT_A2],
                        mybir.ActivationFunctionType.Square,
                        accum_out=ssa2[:],
                    )
                    # combine the early partials off the critical path (DVE)
                    sab = small.tile([P, 1], F32, name="sab", tag="sab")
                    nc.vector.tensor_add(sab[:], ssa1[:], ssa2[:])
                    sabb = small.tile([P, 1], F32, name="sabb", tag="sabb")
                    nc.vector.tensor_add(sabb[:], sab[:], ssb[:])
                    # final chunk: the only norm work after the stream ends
                    sqc = scratch.tile([P, D - LT_DV], F32, name="sqc", tag="sqc")
                    ssc = small.tile([P, 1], F32, name="ssc", tag="ssc")
                    nc.scalar.activation(
                        sqc[:],
                        xtf[:, LT_DV:D],
                        mybir.ActivationFunctionType.Square,
                        accum_out=ssc[:],
                    )
                    nrm = small.tile([P, 1], F32, name="nrm", tag="nrm")
                    nc.scalar.activation(
                        nrm[:],
                        ssc[:],
                        mybir.ActivationFunctionType.Sqrt,
                        bias=sabb[:],
                    )

                inv = small.tile([P, 1], xt_dt, name="inv", tag="inv")
                if MM_F32R:
                    with nc.allow_low_precision(reason="f32r keeps f32 width"):
                        nc.vector.reciprocal(inv[:], nrm[:])
                else:
                    nc.vector.reciprocal(inv[:], nrm[:])

                for k in range(N_CHUNKS):
                    nc.tensor.matmul(
                        psums[k][:],
                        lhsT=inv[:],
                        rhs=xt[:, k * N_CHUNK : (k + 1) * N_CHUNK],
                        start=(t == 0),
                        stop=(t == N_TILES - 1),
                    )

            # drain PSUM: per-bank copies gated only on that bank's stop
            # matmul, alternating ACT/DVE so they stagger with the matmuls
            for k in range(N_CHUNKS):
                dst = s_sb[:, k * N_CHUNK : (k + 1) * N_CHUNK]
                if k % 2 == 0:
                    nc.scalar.copy(dst, psums[k][:])
                else:
                    nc.vector.tensor_copy(dst, psums[k][:])
            nc.sync.dma_start(out=s_out[:], in_=s_sb[:])

    # Full bacc lowering: splits multi-sem waits into event semaphores,
    # moves matmul waits onto LDWEIGHTS, populates extended-inst ISA bytes.
    nc.compile()
    return nc


def get_nc():
    if "nc" not in _CACHE:
        _CACHE["nc"] = _build_nc()
    return _CACHE["nc"]


def make_in_maps(h):
    flat = np.ascontiguousarray(np.asarray(h, dtype=np.float32)).reshape(B * S, D)
    return [
        {"x": flat[c * ROWS_PER_CORE : (c + 1) * ROWS_PER_CORE]}
        for c in range(N_CORES)
    ]


def finish(results, alpha, beta):
    """Combine per-core partial outputs (host, float64)."""
    s_parts = np.stack([np.asarray(r["s_out"][0], dtype=np.float64) for r in results])
    cores_per_batch = N_CORES // B
    s_b = s_parts.reshape(B, cores_per_batch, D).sum(axis=1)  # (B, D)
    sum_sim = float((s_b * s_b).sum())
    # trace(sim) = B*S exactly: each u_i is unit-norm (||h_i|| ~ sqrt(D),
    # never within 1e9x of eps), so u_i.u_i = 1 in real arithmetic.
    diag = float(B) * S
    denom = float(B) * S * (S - 1)
    conc = (sum_sim - diag) / denom
    lam = 1.0 / (1.0 + np.exp(-(float(alpha) * (conc - float(beta)))))
    return (
        np.asarray(lam, dtype=np.float32),
        np.asarray(conc, dtype=np.float32),
    )


def kernel(h, alpha, beta):
    import time

    from concourse.bass_utils import run_bass_kernel_spmd

    nc = get_nc()
    in_maps = make_in_maps(h)
    last_err = None
    for attempt in range(3):
        # The axon-tunneled device intermittently reports
        # NRT_EXEC_UNIT_UNRECOVERABLE on an otherwise-healthy NEFF; a
        # short-delay retry recovers it.
        try:
            results = run_bass_kernel_spmd(
                nc, in_maps, core_ids=list(range(N_CORES))
            ).results
            return finish(results, alpha, beta)
        except Exception as e:  # noqa: BLE001 - retry any device-side failure
            last_err = e
            time.sleep(5.0 * (attempt + 1))
    raise last_err


# revision 3
# speedup vs baseline: 1.0207x; 1.0154x over previous
"""CompressionHead kernel for Trainium2 (8 NeuronCores, Bass/Tile).

Reference computes:
    u          = h / max(||h||_2, eps)              (row-normalize, dim=-1)
    sim        = einsum('bid,bjd->bij', u, u)       (B,S,S) batched GEMM
    conc       = (sum(sim) - trace(sim)) / (B*S*(S-1))
    lambda_t   = sigmoid(alpha * (conc - beta))
    returns (lambda_t, conc)

Key identity: sum_{i,j} u_i . u_j = || sum_i u_i ||^2, so the O(B*S^2*D)
GEMM collapses to an O(B*S*D) normalize-and-reduce:
    sum(sim)   = sum_b || s_b ||^2,   s_b = sum_i u_{b,i}
    trace(sim) = sum_{b,i} u_{b,i} . u_{b,i} = B*S exactly (u is unit-norm;
                 ||h|| ~ sqrt(D) >> eps), so it is not computed on device.

Sharding: flatten (B,S) -> 16384 rows; each of the 8 cores takes a
contiguous 2048-row block (2 cores per batch; blocks never straddle a
batch). Per core, per [128, 2048] row-tile:
  - row sum-of-squares split between ACT (Square+accum) and DVE
    (mul+reduce; DVE needs 2 passes so it gets the smaller share)
  - nrm = Sqrt(ssa + bias=ssb) on ACT, inv = 1/nrm on DVE
  - PE matmul psum[1,:] += inv.T @ x accumulated over the 16 row-tiles
    (u never materialized; inv is folded into the MAC). Operands typed
    float32r: full-rate PE (1 cyc/col vs 4 for f32).

The kernel is DMA-bus-bound (16.78MB @ 360GB/s ~ 46.6us); everything else
hides in the stream except the tail. Every DMA's consumer starts >=900ns
(sem propagation) after that DMA lands, so the last ~2us of arrivals
serialize on the engines. The final two tiles therefore stream as
engine-sized column chunks in pipeline-drain order: each ACT chunk's
Square lands its sem just as the previous chunk's Square retires, the
final 128-col chunk goes to DVE (runs in parallel with ACT's last
Square), and the partial-sum combines are pre-folded so Sqrt fires as
soon as the last two partials exist. Then the four stop-matmuls, PSUM ->
SBUF copies interleaved ACT/DVE per bank (each gated only on its own
bank's matmul), and one 8KB output DMA. Host combines the tiny per-core
outputs in float64.
"""

import numpy as np

B, S, D = 4, 4096, 2048
N_CORES = 8
ROWS_PER_CORE = (B * S) // N_CORES  # 2048
P = 128
N_TILES = ROWS_PER_CORE // P  # 16
N_CHUNK = 512  # PSUM-bank / fp32 moving-operand limit per matmul
N_CHUNKS = D // N_CHUNK  # 4

MM_F32R = True  # PE matmul in float32r (full rate vs 4 cyc/row for f32)

_CACHE = {}


def _build_nc():
    import concourse.tile as tile
    from concourse import bacc, mybir

    F32 = mybir.dt.float32
    F32R = mybir.dt.float32r
    SQUARE = mybir.ActivationFunctionType.Square
    SQRT = mybir.ActivationFunctionType.Sqrt
    X = mybir.AxisListType.X
    ADD = mybir.AluOpType.add

    nc = bacc.Bacc(None, target_bir_lowering=False, debug=True)
    x = nc.dram_tensor("x", [ROWS_PER_CORE, D], F32, kind="ExternalInput")
    s_out = nc.dram_tensor("s_out", [1, D], F32, kind="ExternalOutput")

    with tile.TileContext(nc) as tc:
        with (
            tc.tile_pool(name="xp", bufs=4) as xp,
            tc.tile_pool(name="scratch", bufs=2) as scratch,
            tc.tile_pool(name="small", bufs=6) as small,
            tc.tile_pool(name="psum", bufs=1, space="PSUM") as pp,
            tc.tile_pool(name="outp", bufs=1) as outp,
        ):
            psums = [
                pp.tile([1, N_CHUNK], F32, name=f"ps{k}", tag=f"ps{k}")
                for k in range(N_CHUNKS)
            ]
            s_sb = outp.tile([1, D], F32, name="s_sb")

            xt_dt = F32R if MM_F32R else F32

            def new_xt():
                return xp.tile([P, D], xt_dt, name="xt", tag="xt")

            def dma_cols(xt, t, c0, c1):
                src = x[t * P : (t + 1) * P, c0:c1]
                if MM_F32R:
                    src = src.bitcast(F32R)
                nc.sync.dma_start(out=xt[:, c0:c1], in_=src)

            def act_sq(xtf, c0, c1, tag):
                """ACT: Square cols [c0:c1), accumulate row-sum into a [P,1]."""
                sq = scratch.tile([P, c1 - c0], F32, name=f"sq_{tag}", tag="sqs")
                ss = small.tile([P, 1], F32, name=f"ss_{tag}", tag=f"ss_{tag}")
                nc.scalar.activation(sq[:], xtf[:, c0:c1], SQUARE, accum_out=ss[:])
                return ss

            def dve_sq(xtf, c0, c1, tag):
                """DVE: mul+reduce cols [c0:c1) into a [P,1]."""
                sq = scratch.tile([P, c1 - c0], F32, name=f"dq_{tag}", tag="dqs")
                nc.vector.tensor_mul(sq[:], xtf[:, c0:c1], xtf[:, c0:c1])
                ss = small.tile([P, 1], F32, name=f"ds_{tag}", tag=f"ds_{tag}")
                nc.vector.tensor_reduce(ss[:], sq[:], axis=X, op=ADD)
                return ss

            def dve_add(a, b, tag):
                o = small.tile([P, 1], F32, name=f"ad_{tag}", tag=f"ad_{tag}")
                nc.vector.tensor_add(o[:], a[:], b[:])
                return o

            def sqrt_recip(ss_main, ss_bias, tag):
                """nrm = Sqrt(ss_main + ss_bias) on ACT; inv = 1/nrm on DVE."""
                nrm = small.tile([P, 1], F32, name=f"nrm_{tag}", tag=f"nrm_{tag}")
                nc.scalar.activation(nrm[:], ss_main[:], SQRT, bias=ss_bias[:])
                inv = small.tile([P, 1], xt_dt, name=f"inv_{tag}", tag=f"inv_{tag}")
                if MM_F32R:
                    with nc.allow_low_precision(reason="f32r keeps f32 width"):
                        nc.vector.reciprocal(inv[:], nrm[:])
                else:
                    nc.vector.reciprocal(inv[:], nrm[:])
                return inv

            def matmuls(inv, xt, t):
                for k in range(N_CHUNKS):
                    nc.tensor.matmul(
                        psums[k][:],
                        lhsT=inv[:],
                        rhs=xt[:, k * N_CHUNK : (k + 1) * N_CHUNK],
                        start=(t == 0),
                        stop=(t == N_TILES - 1),
                    )

            # --- tiles 0..13: one full-tile DMA each, ACT 1280 / DVE 768 ---
            H = 1280
            for t in range(N_TILES - 2):
                xt = new_xt()
                xtf = xt[:].bitcast(F32) if MM_F32R else xt[:]
                dma_cols(xt, t, 0, D)
                ssa = act_sq(xtf, 0, H, f"t{t}")
                ssb = dve_sq(xtf, H, D, f"t{t}")
                inv = sqrt_recip(ssa, ssb, f"t{t}")
                matmuls(inv, xt, t)

            # --- tiles 14/15: column chunks in pipeline-drain bus order ---
            # (widths chosen so each ACT chunk's 900ns DMA-sem lands just as
            # the previous chunk's Square retires; DVE absorbs the rest and
            # the final 128-col chunk, which runs in parallel with ACT's
            # last Square.)
            t14, t15 = N_TILES - 2, N_TILES - 1
            x14, x15 = new_xt(), new_xt()
            x14f = x14[:].bitcast(F32) if MM_F32R else x14[:]
            x15f = x15[:].bitcast(F32) if MM_F32R else x15[:]

            # bus order: t14[1536:2048]D, t14[0:640]A, t14[640:1280]A,
            #            t15[1280:1920]D, t14[1280:1536]A, t15[0:640]A,
            #            t15[640:1280]A, t15[1920:2048]D(last)
            dma_cols(x14, t14, 1536, 2048)
            dma_cols(x14, t14, 0, 640)
            dma_cols(x14, t14, 640, 1280)
            dma_cols(x15, t15, 1280, 1920)
            dma_cols(x14, t14, 1280, 1536)
            dma_cols(x15, t15, 0, 640)
            dma_cols(x15, t15, 640, 1280)
            dma_cols(x15, t15, 1920, 2048)

            # t14 norm: ACT [0:640]+[640:1280]+[1280:1536], DVE [1536:2048]
            sb14 = dve_sq(x14f, 1536, 2048, "b14")
            sa14_1 = act_sq(x14f, 0, 640, "a14_1")
            sa14_2 = act_sq(x14f, 640, 1280, "a14_2")
            c14_1 = dve_add(sa14_1, sa14_2, "c14_1")
            c14_2 = dve_add(c14_1, sb14, "c14_2")
            sa14_3 = act_sq(x14f, 1280, 1536, "a14_3")
            inv14 = sqrt_recip(sa14_3, c14_2, "t14")
            matmuls(inv14, x14, t14)

            # t15 norm: DVE [1280:1920], ACT [0:640]+[640:1280],
            # DVE [1920:2048] (the final chunk, parallel with ACT's last sq)
            sb15 = dve_sq(x15f, 1280, 1920, "b15")
            sa15_1 = act_sq(x15f, 0, 640, "a15_1")
            c15_1 = dve_add(sa15_1, sb15, "c15_1")
            sa15_2 = act_sq(x15f, 640, 1280, "a15_2")
            ssc = dve_sq(x15f, 1920, 2048, "c15")
            c15_2 = dve_add(c15_1, ssc, "c15_2")
            inv15 = sqrt_recip(sa15_2, c15_2, "t15")
            matmuls(inv15, x15, t15)

            # drain PSUM: per-bank copies gated only on that bank's stop
            # matmul, alternating ACT/DVE so they stagger with the matmuls
            for k in range(N_CHUNKS):
                dst = s_sb[:, k * N_CHUNK : (k + 1) * N_CHUNK]
                if k % 2 == 0:
                    nc.scalar.copy(dst, psums[k][:])
                else:
                    nc.vector.tensor_copy(dst, psums[k][:])
            nc.sync.dma_start(out=s_out[:], in_=s_sb[:])

    # Full bacc lowering: splits multi-sem waits into event semaphores,
    # moves matmul waits onto LDWEIGHTS, populates extended-inst ISA bytes.
    nc.compile()
    return nc


def get_nc():
    if "nc" not in _CACHE:
        _CACHE["nc"] = _build_nc()
    return _CACHE["nc"]


def make_in_maps(h):
    flat = np.ascontiguousarray(np.asarray(h, dtype=np.float32)).reshape(B * S, D)
    return [
        {"x": flat[c * ROWS_PER_CORE : (c + 1) * ROWS_PER_CORE]}
        for c in range(N_CORES)
    ]


def finish(results, alpha, beta):
    """Combine per-core partial outputs (host, float64)."""
    s_parts = np.stack([np.asarray(r["s_out"][0], dtype=np.float64) for r in results])
    cores_per_batch = N_CORES // B
    s_b = s_parts.reshape(B, cores_per_batch, D).sum(axis=1)  # (B, D)
    sum_sim = float((s_b * s_b).sum())
    # trace(sim) = B*S exactly: each u_i is unit-norm (||h_i|| ~ sqrt(D),
    # never within 1e9x of eps), so u_i.u_i = 1 in real arithmetic.
    diag = float(B) * S
    denom = float(B) * S * (S - 1)
    conc = (sum_sim - diag) / denom
    lam = 1.0 / (1.0 + np.exp(-(float(alpha) * (conc - float(beta)))))
    return (
        np.asarray(lam, dtype=np.float32),
        np.asarray(conc, dtype=np.float32),
    )


def kernel(h, alpha, beta):
    import time

    from concourse.bass_utils import run_bass_kernel_spmd

    nc = get_nc()
    in_maps = make_in_maps(h)
    last_err = None
    for attempt in range(3):
        # The axon-tunneled device intermittently reports
        # NRT_EXEC_UNIT_UNRECOVERABLE on an otherwise-healthy NEFF; a
        # short-delay retry recovers it.
        try:
            results = run_bass_kernel_spmd(
                nc, in_maps, core_ids=list(range(N_CORES))
            ).results
            return finish(results, alpha, beta)
        except Exception as e:  # noqa: BLE001 - retry any device-side failure
            last_err = e
            time.sleep(5.0 * (attempt + 1))
    raise last_err
